# revision 8
# baseline (speedup 1.0000x reference)
"""Trainium2 Bass kernel for nn_MixedLoraModel_734.

Computes, for T=8192 tokens, D=4096:
    out = x @ W_base^T + b_base + scaling[token_lora][:,None] * lora(x)
where lora(x)[t] = WB[l_t] @ (WA[l_t] @ x[t]),  l_t = token_lora[t],
L=8 adapters of rank R=16 (so the full adapter stack is 8*16 = 128 rows).

Strategy (8 NeuronCores, data-parallel over tokens):
  - Each core gets a 1024-token shard of x / token_lora plus replicated
    W_base, b_base, WA, WB, scaling.
  - Routing is done densely with zero data-dependent control flow:
      u_allT[j, t] = sum_d WA_flat[j, d] * x[t, d]          (j = 16*l + r)
      maskT[j, t]  = (j // 16 == token_lora[t])
      u_mT         = u_allT * maskT
      v[t, o]      = sum_j u_mT[j, t] * (scaling[j//16] * WB[j//16, o, j%16])
    Since u_mT is zero outside each token's own adapter block, one dense
    K=128 matmul applies the per-token selected adapter.
  - The base matmul accumulates in PSUM per (token-tile, 256-wide o-chunk);
    the v matmul accumulates into the same PSUM tiles (start=False), then a
    single eviction adds the bias and DMAs out.
  - All matmul operands use float32r (full-rate fp32 PE mode, N>=256).
  - W_base / x / WA / WB are transposed on-chip with PE-transposes (the
    contraction dim must live on SBUF partitions).
"""

import numpy as np

import concourse.bass as bass
import concourse.mybir as mybir
import concourse.tile as tile
from concourse import bacc
from concourse.masks import make_identity

P = 128
D = 4096          # d_in
O = 4096          # d_out
NCORES = 8
T = 8192
TS = T // NCORES  # 1024 tokens per core
NT = TS // P      # 8 token tiles per core
ND = D // P       # 32 contraction chunks
OC = 256          # output-chunk width (PE moving-dim; >=256 keeps f32r at full rate)
NOC = O // OC     # 16
L, R, LR = 8, 16, 128

F32 = mybir.dt.float32
F32R = mybir.dt.float32r
I32 = mybir.dt.int32
EQ = mybir.AluOpType.is_equal
MUL = mybir.AluOpType.mult


def _build() -> bass.Bass:
    nc = bacc.Bacc(None)

    x = nc.declare_dram_parameter("x", [TS, D], F32, isOutput=False)
    w = nc.declare_dram_parameter("w", [O, D], F32, isOutput=False)
    b = nc.declare_dram_parameter("b", [O], F32, isOutput=False)
    wa = nc.declare_dram_parameter("wa", [LR, D], F32, isOutput=False)
    wb = nc.declare_dram_parameter("wb", [L, O, R], F32, isOutput=False)
    scal = nc.declare_dram_parameter("scal", [L], F32, isOutput=False)
    tl = nc.declare_dram_parameter("tl", [TS], I32, isOutput=False)
    out = nc.declare_dram_parameter("out", [TS, O], F32, isOutput=True)

    with tile.TileContext(nc) as tc:
        with (
            tc.tile_pool(name="const", bufs=1) as const,
            tc.tile_pool(name="res", bufs=1) as res,
        ):
            ident = const.tile([P, P], F32)
            make_identity(nc, ident)
            ones_row = const.tile([1, P], F32)
            nc.vector.memset(ones_row[:], 1.0)
            jdiv16 = const.tile([P, 1], F32)

            # Resident operand stacks (all float32r, fed only to the PE):
            # xT:   chunk dc occupies cols [dc*TS, (dc+1)*TS);
            #       xT[p, dc*TS + t] = x[t, dc*128 + p]
            # wbsT: wbsT[j, o] = scaling[j//16] * WB[j//16, o, j%16]
            # u_mT: u_mT[j, t] = masked, per-token-selected  x[t] @ WA[l_t]^T
            xT = res.tile([P, ND * TS], F32R, tag="xT")
            wbsT = res.tile([P, O], F32R, tag="wbsT")
            u_mT = res.tile([P, TS], F32R, tag="u_mT")

            # ---------------- prepass ----------------
            # Phase S: index/scaling columns.
            with (
                tc.tile_pool(name="preS", bufs=1) as preS,
                tc.tile_pool(name="psS", bufs=1, space="PSUM") as psS,
            ):
                scal16 = preS.tile([P, 1], F32, tag="s16")

                irow_i = preS.tile([1, P], I32, tag="iri")
                nc.gpsimd.iota(irow_i[:], pattern=[[1, L], [0, R]], base=0,
                               channel_multiplier=0)
                irow_f = preS.tile([1, P], F32, tag="irf")
                nc.vector.tensor_copy(irow_f[:], irow_i[:])
                pcol = psS.tile([P, 1], F32, tag="pcol")
                nc.tensor.matmul(pcol[:], irow_f[:], ones_row[0:1, 0:1],
                                 start=True, stop=True)
                nc.vector.tensor_copy(jdiv16[:], pcol[:])

                # scal16[p] = scaling[p//16] via E[l, j] = (j//16 == l):
                # scal16 = E^T @ scaling
                scal_sb = preS.tile([L, 1], F32, tag="ssb")
                nc.sync.dma_start(out=scal_sb[:],
                                  in_=scal.rearrange("(p f) -> p f", f=1))
                lcol_i = preS.tile([L, 1], I32, tag="lci")
                nc.gpsimd.iota(lcol_i[:], pattern=[[0, 1]], base=0,
                               channel_multiplier=1)
                lcol_f = preS.tile([L, 1], F32, tag="lcf")
                nc.vector.tensor_copy(lcol_f[:], lcol_i[:])
                ibc8 = psS.tile([L, P], F32, tag="ibc8")
                nc.tensor.matmul(ibc8[:], ones_row[0:1, 0:L], irow_f[:],
                                 start=True, stop=True)
                e_sb = preS.tile([L, P], F32, tag="esb")
                nc.vector.tensor_scalar(e_sb[:], ibc8[:], lcol_f[:], None, EQ)
                s16ps = psS.tile([P, 1], F32, tag="s16ps")
                nc.tensor.matmul(s16ps[:], e_sb[:], scal_sb[:],
                                 start=True, stop=True)
                nc.vector.tensor_copy(scal16[:], s16ps[:])

                # Phase W: adapters. wbsT: per 128-wide o-tile, one DMA gathers
                # [o=128, (l,r)=128], one PE transpose flips to [(l,r), o], and
                # the psum->sbuf eviction folds in scaling while rounding f32r.
                waT = preS.tile([P, D], F32R, tag="waT")
                with (
                    tc.tile_pool(name="preW", bufs=2) as preW,
                    tc.tile_pool(name="psW", bufs=2, space="PSUM") as psW,
                ):
                    for ot in range(O // P):
                        nat = preW.tile([P, P], F32, tag="wbnat")
                        src = wb[:, ot * P:(ot + 1) * P, :].transpose([1, 0, 2])
                        nc.sync.dma_start(out=nat[:], in_=src)
                        pt = psW.tile([P, P], F32, tag="wbps")
                        nc.tensor.transpose(pt[:], nat[:], ident[:])
                        nc.vector.tensor_scalar(wbsT[:, ot * P:(ot + 1) * P],
                                                pt[:], scal16[:], None, MUL)

                    # WA -> WAT chunks [d, j]
                    for q in range(4):
                        wa_nat = preW.tile([P, 1024], F32, tag="nat1k")
                        nc.sync.dma_start(out=wa_nat[:],
                                          in_=wa[:, q * 1024:(q + 1) * 1024])
                        for k in range(8):
                            dc = q * 8 + k
                            pt = psW.tile([P, P], F32, tag="waps")
                            nc.tensor.transpose(pt[:], wa_nat[:, k * P:(k + 1) * P],
                                                ident[:])
                            nc.any.tensor_copy(waT[:, dc * P:(dc + 1) * P], pt[:])

                # Phase X: x -> xT (PE transposes, batched 4 per PSUM bank)
                with (
                    tc.tile_pool(name="preX", bufs=3) as preX,
                    tc.tile_pool(name="psX", bufs=3, space="PSUM") as psX,
                ):
                    for tt in range(NT):
                        for q in range(4):
                            x_nat = preX.tile([P, 1024], F32, tag="nat1k")
                            nc.sync.dma_start(
                                out=x_nat[:],
                                in_=x[tt * P:(tt + 1) * P,
                                      q * 1024:(q + 1) * 1024])
                            for half in range(2):
                                pt = psX.tile([P, 4 * P], F32, tag="xps")
                                for k in range(4):
                                    kk = half * 4 + k
                                    nc.tensor.transpose(
                                        pt[:, k * P:(k + 1) * P],
                                        x_nat[:, kk * P:(kk + 1) * P],
                                        ident[:])
                                for k in range(4):
                                    dc = q * 8 + half * 4 + k
                                    nc.any.tensor_copy(
                                        xT[:, dc * TS + tt * P:
                                           dc * TS + (tt + 1) * P],
                                        pt[:, k * P:(k + 1) * P])

                # Phase U: u_allT + routing mask -> u_mT (two 512-token groups)
                with (
                    tc.tile_pool(name="preU", bufs=2) as preU,
                    tc.tile_pool(name="psU", bufs=2, space="PSUM") as psU,
                ):
                    for g in range(2):
                        t0 = g * 512
                        tli = preU.tile([1, 512], I32, tag="tli")
                        nc.sync.dma_start(
                            out=tli[:],
                            in_=tl[t0:t0 + 512].rearrange("(a f) -> a f", a=1))
                        tlf = preU.tile([1, 512], F32, tag="tlf")
                        nc.vector.tensor_copy(tlf[:], tli[:])
                        tlbc = psU.tile([P, 512], F32, tag="tlbc")
                        nc.tensor.matmul(tlbc[:], ones_row[:], tlf[:],
                                         start=True, stop=True)
                        maskT = preU.tile([P, 512], F32, tag="maskT")
                        nc.vector.tensor_scalar(maskT[:], tlbc[:], jdiv16[:],
                                                None, EQ)
                        ups = psU.tile([P, 512], F32, tag="ups")
                        for dc in range(ND):
                            nc.tensor.matmul(
                                ups[:],
                                waT[:, dc * P:(dc + 1) * P],
                                xT[:, dc * TS + t0: dc * TS + t0 + 512],
                                start=(dc == 0), stop=(dc == ND - 1))
                        nc.vector.tensor_tensor(u_mT[:, t0:t0 + 512], ups[:],
                                                maskT[:], MUL)

            # ---------------- main loop ----------------
            with (
                tc.tile_pool(name="wnat", bufs=5) as wnat_p,
                tc.tile_pool(name="wt", bufs=4) as wt_p,
                tc.tile_pool(name="outp", bufs=4) as out_p,
                tc.tile_pool(name="biasp", bufs=2) as bias_p,
                tc.tile_pool(name="acc_ps", bufs=4, space="PSUM") as acc_ps,
                tc.tile_pool(name="tr_ps", bufs=2, space="PSUM") as tr_ps,
                tc.tile_pool(name="b_ps", bufs=1, space="PSUM") as b_ps,
            ):
                for oc in range(NOC):
                    o0 = oc * OC

                    accs = [acc_ps.tile([P, 512], F32, tag="acc",
                                        name=f"acc{oc}_{g}") for g in range(4)]

                    for dq in range(4):
                        # W rows for this o-chunk / d-quarter: 2 o-subtiles
                        wn = []
                        for os_ in range(2):
                            wtile = wnat_p.tile([P, 1024], F32, tag="wn",
                                                name=f"wn{oc}_{dq}_{os_}")
                            nc.sync.dma_start(
                                out=wtile[:],
                                in_=w[o0 + os_ * P: o0 + (os_ + 1) * P,
                                      dq * 1024:(dq + 1) * 1024])
                            wn.append(wtile)

                        for dr in range(8):
                            dc = dq * 8 + dr
                            pt = tr_ps.tile([P, OC], F32, tag="wtps")
                            for os_ in range(2):
                                nc.tensor.transpose(
                                    pt[:, os_ * P:(os_ + 1) * P],
                                    wn[os_][:, dr * P:(dr + 1) * P],
                                    ident[:])
                            wt = wt_p.tile([P, OC], F32R, tag="wt")
                            nc.any.tensor_copy(wt[:], pt[:])
                            for g in range(4):
                                for h in range(2):
                                    tt = 2 * g + h
                                    nc.tensor.matmul(
                                        accs[g][:, h * OC:(h + 1) * OC],
                                        xT[:, dc * TS + tt * P:
                                           dc * TS + (tt + 1) * P],
                                        wt[:],
                                        start=(dc == 0 and h == 0),
                                        stop=False)

                    # LoRA contribution accumulates into the same PSUM tiles
                    for g in range(4):
                        for h in range(2):
                            tt = 2 * g + h
                            nc.tensor.matmul(
                                accs[g][:, h * OC:(h + 1) * OC],
                                u_mT[:, tt * P:(tt + 1) * P],
                                wbsT[:, o0:o0 + OC],
                                start=False, stop=(h == 1))

                    # bias broadcast for this o-chunk
                    brow = bias_p.tile([1, OC], F32, tag="brow")
                    nc.sync.dma_start(
                        out=brow[:],
                        in_=b[o0:o0 + OC].rearrange("(a f) -> a f", a=1))
                    bps = b_ps.tile([P, OC], F32, tag="bps")
                    nc.tensor.matmul(bps[:], ones_row[:], brow[:],
                                     start=True, stop=True)
                    bias_sb = bias_p.tile([P, OC], F32, tag="bias")
                    nc.any.tensor_copy(bias_sb[:], bps[:])

                    for g in range(4):
                        for h in range(2):
                            tt = 2 * g + h
                            osb = out_p.tile([P, OC], F32, tag="osb",
                                             name=f"osb{oc}_{g}_{h}")
                            nc.vector.tensor_add(
                                osb[:], accs[g][:, h * OC:(h + 1) * OC],
                                bias_sb[:])
                            nc.sync.dma_start(
                                out=out[tt * P:(tt + 1) * P, o0:o0 + OC],
                                in_=osb[:])
    nc.finalize()
    return nc


_NC = None


def _get_nc():
    global _NC
    if _NC is None:
        _NC = _build()
    return _NC


class _Runner:
    """Cached PJRT executable for the SPMD bass kernel.

    Mirrors concourse.bass2jax.run_bass_via_pjrt's multi-core path but
    keeps the jitted shard_map callable alive across invocations so
    repeated kernel() calls skip retrace/recompile.
    """

    # inputs sharded over the token dim; everything else replicated
    _CORE_SHARDED = {"x", "tl"}

    def __init__(self):
        import jax
        import concourse.mybir as mybir_
        from concourse import bass2jax

        bass2jax.install_neuronx_cc_hook()
        self._bass2jax = bass2jax
        nc = _get_nc()
        self.nc = nc

        partition_name = (nc.partition_id_tensor.name
                          if nc.partition_id_tensor else None)
        in_names, out_names, out_avals, zero_outs = [], [], [], []
        for alloc in nc.m.functions[0].allocations:
            if not isinstance(alloc, mybir_.MemoryLocationSet):
                continue
            name = alloc.memorylocations[0].name
            if alloc.kind == "ExternalInput":
                if name != partition_name:
                    in_names.append(name)
            elif alloc.kind == "ExternalOutput":
                shape = tuple(alloc.tensor_shape)
                dtype = mybir_.dt.np(alloc.dtype)
                out_names.append(name)
                out_avals.append(jax.core.ShapedArray(shape, dtype))
                zero_outs.append((shape, dtype))
        self.in_names = list(in_names)
        self.out_names = out_names
        self.out_avals = out_avals
        n_params = len(in_names)
        all_in_names = in_names + out_names
        if partition_name is not None:
            all_in_names.append(partition_name)

        from jax.experimental.shard_map import shard_map
        from jax.sharding import Mesh, NamedSharding, PartitionSpec

        devices = jax.devices()[:NCORES]
        assert len(devices) == NCORES, devices
        mesh = Mesh(np.asarray(devices), ("core",))
        self.mesh = mesh

        def spec_for(name):
            return (PartitionSpec("core") if name in self._CORE_SHARDED
                    else PartitionSpec())

        in_specs = tuple(spec_for(n) for n in in_names) + \
            (PartitionSpec("core"),) * len(out_names)
        out_specs = (PartitionSpec("core"),) * len(out_names)
        self.in_shardings = [NamedSharding(mesh, spec_for(n))
                             for n in in_names]
        self.out_sharding = NamedSharding(mesh, PartitionSpec("core"))

        def _body(*args):
            operands = list(args)
            if partition_name is not None:
                operands.append(bass2jax.partition_id_tensor())
            outs = bass2jax._bass_exec_p.bind(
                *operands,
                out_avals=tuple(out_avals),
                in_names=tuple(all_in_names),
                out_names=tuple(out_names),
                lowering_input_output_aliases=(),
                sim_require_finite=True,
                sim_require_nnan=True,
                nc=nc,
            )
            return tuple(outs)

        donate = tuple(range(n_params, n_params + len(out_names)))
        self._fn = jax.jit(
            shard_map(_body, mesh=mesh, in_specs=in_specs,
                      out_specs=out_specs, check_rep=False),
            donate_argnums=donate, keep_unused=True)
        # donated output scratch (regenerated from the previous call's
        # result; the kernel writes every element so contents are moot)
        self._scratch = [
            np.zeros((NCORES * s[0], *s[1:]), dt) for (s, dt) in
            [((a.shape[0], *a.shape[1:]), a.dtype) for a in out_avals]
        ]

    def put_inputs(self, by_name):
        import jax
        out = []
        for name, sharding in zip(self.in_names, self.in_shardings):
            out.append(jax.device_put(by_name[name], sharding))
        return out

    def run_device(self, dev_args):
        """dev_args: device arrays in in_names order. Returns jax arrays."""
        import jax
        scratch = [jax.device_put(z, self.out_sharding)
                   for z in self._scratch]
        outs = self._fn(*dev_args, *scratch)
        return outs

    def run(self, by_name):
        outs = self.run_device(self.put_inputs(by_name))
        host = [np.asarray(o) for o in outs]
        # feed this call's (now host-copied) outputs back as the next
        # call's donated scratch
        self._scratch = host
        return {n: h for n, h in zip(self.out_names, host)}


_RUNNER = None


def _get_runner():
    global _RUNNER
    if _RUNNER is None:
        _RUNNER = _Runner()
    return _RUNNER


def _global_inputs(x, W_base, b_base, WA, WB, scaling, token_lora):
    """Full-size (global) arrays keyed by DRAM-parameter name."""
    return {
        "x": np.ascontiguousarray(np.asarray(x, dtype=np.float32)),
        "w": np.ascontiguousarray(np.asarray(W_base, dtype=np.float32)),
        "b": np.ascontiguousarray(np.asarray(b_base, dtype=np.float32)),
        "wa": np.ascontiguousarray(
            np.asarray(WA, dtype=np.float32).reshape(LR, D)),
        "wb": np.ascontiguousarray(np.asarray(WB, dtype=np.float32)),
        "scal": np.ascontiguousarray(np.asarray(scaling, dtype=np.float32)),
        "tl": np.ascontiguousarray(np.asarray(token_lora, dtype=np.int32)),
    }


def kernel(x, W_base, b_base, WA, WB, scaling, token_lora):
    by_name = _global_inputs(x, W_base, b_base, WA, WB, scaling, token_lora)
    try:
        res = _get_runner().run(by_name)
        return res["out"]
    except Exception:
        # robust fallback through the library SPMD path
        from concourse.bass_utils import run_bass_kernel_spmd

        nc = _get_nc()
        in_maps = []
        for c in range(NCORES):
            in_maps.append({
                "x": by_name["x"][c * TS:(c + 1) * TS],
                "w": by_name["w"],
                "b": by_name["b"],
                "wa": by_name["wa"],
                "wb": by_name["wb"],
                "scal": by_name["scal"],
                "tl": by_name["tl"][c * TS:(c + 1) * TS],
            })
        res = run_bass_kernel_spmd(nc, in_maps, core_ids=list(range(NCORES)))
        return np.concatenate(
            [res.results[c]["out"] for c in range(NCORES)], axis=0)


# revision 9
# speedup vs baseline: 123.9922x; 123.9922x over previous
"""Trainium2 Bass kernel for nn_MixedLoraModel_734.

Computes, for T=8192 tokens, D=4096:
    out = x @ W_base^T + b_base + scaling[token_lora][:,None] * lora(x)
where lora(x)[t] = WB[l_t] @ (WA[l_t] @ x[t]),  l_t = token_lora[t],
L=8 adapters of rank R=16 (so the full adapter stack is 8*16 = 128 rows).

Strategy (8 NeuronCores, data-parallel over tokens):
  - Each core gets a 1024-token shard of x / token_lora plus replicated
    W_base, b_base, WA, WB, scaling.
  - Routing is done densely with zero data-dependent control flow:
      u_allT[j, t] = sum_d WA_flat[j, d] * x[t, d]          (j = 16*l + r)
      maskT[j, t]  = (j // 16 == token_lora[t])
      u_mT         = u_allT * maskT
      v[t, o]      = sum_j u_mT[j, t] * (scaling[j//16] * WB[j//16, o, j%16])
    Since u_mT is zero outside each token's own adapter block, one dense
    K=128 matmul applies the per-token selected adapter.
  - The base matmul accumulates in PSUM per (token-tile, 256-wide o-chunk);
    the v matmul accumulates into the same PSUM tiles (start=False), then a
    single eviction adds the bias and DMAs out.
  - All matmul operands use float32r (full-rate fp32 PE mode, N>=256).
  - W_base / x / WA / WB are transposed on-chip with PE-transposes (the
    contraction dim must live on SBUF partitions).
"""

import numpy as np

import concourse.bass as bass
import concourse.mybir as mybir
import concourse.tile as tile
from concourse import bacc
from concourse.masks import make_identity

P = 128
D = 4096          # d_in
O = 4096          # d_out
NCORES = 8
T = 8192
TS = T // NCORES  # 1024 tokens per core
NT = TS // P      # 8 token tiles per core
ND = D // P       # 32 contraction chunks
OC = 256          # output-chunk width (PE moving-dim; >=256 keeps f32r at full rate)
NOC = O // OC     # 16
L, R, LR = 8, 16, 128

F32 = mybir.dt.float32
F32R = mybir.dt.float32r
I32 = mybir.dt.int32
EQ = mybir.AluOpType.is_equal
MUL = mybir.AluOpType.mult


def _build() -> bass.Bass:
    nc = bacc.Bacc(None)

    x = nc.declare_dram_parameter("x", [TS, D], F32, isOutput=False)
    w = nc.declare_dram_parameter("w", [O, D], F32, isOutput=False)
    b = nc.declare_dram_parameter("b", [O], F32, isOutput=False)
    wa = nc.declare_dram_parameter("wa", [LR, D], F32, isOutput=False)
    wb = nc.declare_dram_parameter("wb", [L, O, R], F32, isOutput=False)
    scal = nc.declare_dram_parameter("scal", [L], F32, isOutput=False)
    tl = nc.declare_dram_parameter("tl", [TS], I32, isOutput=False)
    out = nc.declare_dram_parameter("out", [TS, O], F32, isOutput=True)

    with tile.TileContext(nc) as tc:
        with (
            tc.tile_pool(name="const", bufs=1) as const,
            tc.tile_pool(name="res", bufs=1) as res,
        ):
            ident = const.tile([P, P], F32)
            make_identity(nc, ident)
            ones_row = const.tile([1, P], F32)
            nc.vector.memset(ones_row[:], 1.0)
            jdiv16 = const.tile([P, 1], F32)

            # Resident operand stacks (all float32r, fed only to the PE):
            # xT:   chunk dc occupies cols [dc*TS, (dc+1)*TS);
            #       xT[p, dc*TS + t] = x[t, dc*128 + p]
            # wbsT: wbsT[j, o] = scaling[j//16] * WB[j//16, o, j%16]
            # u_mT: u_mT[j, t] = masked, per-token-selected  x[t] @ WA[l_t]^T
            xT = res.tile([P, ND * TS], F32R, tag="xT")
            wbsT = res.tile([P, O], F32R, tag="wbsT")
            u_mT = res.tile([P, TS], F32R, tag="u_mT")

            # ---------------- prepass ----------------
            # Phase S: index/scaling columns.
            with (
                tc.tile_pool(name="preS", bufs=1) as preS,
                tc.tile_pool(name="psS", bufs=1, space="PSUM") as psS,
            ):
                scal16 = preS.tile([P, 1], F32, tag="s16")

                irow_i = preS.tile([1, P], I32, tag="iri")
                nc.gpsimd.iota(irow_i[:], pattern=[[1, L], [0, R]], base=0,
                               channel_multiplier=0)
                irow_f = preS.tile([1, P], F32, tag="irf")
                nc.vector.tensor_copy(irow_f[:], irow_i[:])
                pcol = psS.tile([P, 1], F32, tag="pcol")
                nc.tensor.matmul(pcol[:], irow_f[:], ones_row[0:1, 0:1],
                                 start=True, stop=True)
                nc.vector.tensor_copy(jdiv16[:], pcol[:])

                # scal16[p] = scaling[p//16] via E[l, j] = (j//16 == l):
                # scal16 = E^T @ scaling
                scal_sb = preS.tile([L, 1], F32, tag="ssb")
                nc.sync.dma_start(out=scal_sb[:],
                                  in_=scal.rearrange("(p f) -> p f", f=1))
                lcol_i = preS.tile([L, 1], I32, tag="lci")
                nc.gpsimd.iota(lcol_i[:], pattern=[[0, 1]], base=0,
                               channel_multiplier=1)
                lcol_f = preS.tile([L, 1], F32, tag="lcf")
                nc.vector.tensor_copy(lcol_f[:], lcol_i[:])
                ibc8 = psS.tile([L, P], F32, tag="ibc8")
                nc.tensor.matmul(ibc8[:], ones_row[0:1, 0:L], irow_f[:],
                                 start=True, stop=True)
                e_sb = preS.tile([L, P], F32, tag="esb")
                nc.vector.tensor_scalar(e_sb[:], ibc8[:], lcol_f[:], None, EQ)
                s16ps = psS.tile([P, 1], F32, tag="s16ps")
                nc.tensor.matmul(s16ps[:], e_sb[:], scal_sb[:],
                                 start=True, stop=True)
                nc.vector.tensor_copy(scal16[:], s16ps[:])

                # Phase W: adapters. wbsT: per 128-wide o-tile, one DMA gathers
                # [o=128, (l,r)=128], one PE transpose flips to [(l,r), o], and
                # the psum->sbuf eviction folds in scaling while rounding f32r.
                waT = preS.tile([P, D], F32R, tag="waT")
                with (
                    tc.tile_pool(name="preW", bufs=2) as preW,
                    tc.tile_pool(name="psW", bufs=2, space="PSUM") as psW,
                ):
                    for ot in range(O // P):
                        nat = preW.tile([P, P], F32, tag="wbnat")
                        src = wb[:, ot * P:(ot + 1) * P, :].transpose([1, 0, 2])
                        nc.sync.dma_start(out=nat[:], in_=src)
                        pt = psW.tile([P, P], F32, tag="wbps")
                        nc.tensor.transpose(pt[:], nat[:], ident[:])
                        nc.vector.tensor_scalar(wbsT[:, ot * P:(ot + 1) * P],
                                                pt[:], scal16[:], None, MUL)

                    # WA -> WAT chunks [d, j]
                    for q in range(4):
                        wa_nat = preW.tile([P, 1024], F32, tag="nat1k")
                        nc.sync.dma_start(out=wa_nat[:],
                                          in_=wa[:, q * 1024:(q + 1) * 1024])
                        for k in range(8):
                            dc = q * 8 + k
                            pt = psW.tile([P, P], F32, tag="waps")
                            nc.tensor.transpose(pt[:], wa_nat[:, k * P:(k + 1) * P],
                                                ident[:])
                            nc.any.tensor_copy(waT[:, dc * P:(dc + 1) * P], pt[:])

                # Phase X: x -> xT (PE transposes, batched 4 per PSUM bank)
                with (
                    tc.tile_pool(name="preX", bufs=3) as preX,
                    tc.tile_pool(name="psX", bufs=3, space="PSUM") as psX,
                ):
                    for tt in range(NT):
                        for q in range(4):
                            x_nat = preX.tile([P, 1024], F32, tag="nat1k")
                            nc.sync.dma_start(
                                out=x_nat[:],
                                in_=x[tt * P:(tt + 1) * P,
                                      q * 1024:(q + 1) * 1024])
                            for half in range(2):
                                pt = psX.tile([P, 4 * P], F32, tag="xps")
                                for k in range(4):
                                    kk = half * 4 + k
                                    nc.tensor.transpose(
                                        pt[:, k * P:(k + 1) * P],
                                        x_nat[:, kk * P:(kk + 1) * P],
                                        ident[:])
                                for k in range(4):
                                    dc = q * 8 + half * 4 + k
                                    nc.any.tensor_copy(
                                        xT[:, dc * TS + tt * P:
                                           dc * TS + (tt + 1) * P],
                                        pt[:, k * P:(k + 1) * P])

                # Phase U: u_allT + routing mask -> u_mT (two 512-token groups)
                with (
                    tc.tile_pool(name="preU", bufs=2) as preU,
                    tc.tile_pool(name="psU", bufs=2, space="PSUM") as psU,
                ):
                    for g in range(2):
                        t0 = g * 512
                        tli = preU.tile([1, 512], I32, tag="tli")
                        nc.sync.dma_start(
                            out=tli[:],
                            in_=tl[t0:t0 + 512].rearrange("(a f) -> a f", a=1))
                        tlf = preU.tile([1, 512], F32, tag="tlf")
                        nc.vector.tensor_copy(tlf[:], tli[:])
                        tlbc = psU.tile([P, 512], F32, tag="tlbc")
                        nc.tensor.matmul(tlbc[:], ones_row[:], tlf[:],
                                         start=True, stop=True)
                        maskT = preU.tile([P, 512], F32, tag="maskT")
                        nc.vector.tensor_scalar(maskT[:], tlbc[:], jdiv16[:],
                                                None, EQ)
                        ups = psU.tile([P, 512], F32, tag="ups")
                        for dc in range(ND):
                            nc.tensor.matmul(
                                ups[:],
                                waT[:, dc * P:(dc + 1) * P],
                                xT[:, dc * TS + t0: dc * TS + t0 + 512],
                                start=(dc == 0), stop=(dc == ND - 1))
                        nc.vector.tensor_tensor(u_mT[:, t0:t0 + 512], ups[:],
                                                maskT[:], MUL)

            # ---------------- main loop ----------------
            with (
                tc.tile_pool(name="wnat", bufs=5) as wnat_p,
                tc.tile_pool(name="wt", bufs=4) as wt_p,
                tc.tile_pool(name="outp", bufs=4) as out_p,
                tc.tile_pool(name="biasp", bufs=2) as bias_p,
                tc.tile_pool(name="acc_ps", bufs=4, space="PSUM") as acc_ps,
                tc.tile_pool(name="tr_ps", bufs=2, space="PSUM") as tr_ps,
                tc.tile_pool(name="b_ps", bufs=1, space="PSUM") as b_ps,
            ):
                for oc in range(NOC):
                    o0 = oc * OC

                    accs = [acc_ps.tile([P, 512], F32, tag="acc",
                                        name=f"acc{oc}_{g}") for g in range(4)]

                    for dq in range(4):
                        # W rows for this o-chunk / d-quarter: 2 o-subtiles
                        wn = []
                        for os_ in range(2):
                            wtile = wnat_p.tile([P, 1024], F32, tag="wn",
                                                name=f"wn{oc}_{dq}_{os_}")
                            nc.sync.dma_start(
                                out=wtile[:],
                                in_=w[o0 + os_ * P: o0 + (os_ + 1) * P,
                                      dq * 1024:(dq + 1) * 1024])
                            wn.append(wtile)

                        for dr in range(8):
                            dc = dq * 8 + dr
                            pt = tr_ps.tile([P, OC], F32, tag="wtps")
                            for os_ in range(2):
                                nc.tensor.transpose(
                                    pt[:, os_ * P:(os_ + 1) * P],
                                    wn[os_][:, dr * P:(dr + 1) * P],
                                    ident[:])
                            wt = wt_p.tile([P, OC], F32R, tag="wt")
                            nc.any.tensor_copy(wt[:], pt[:])
                            for g in range(4):
                                for h in range(2):
                                    tt = 2 * g + h
                                    nc.tensor.matmul(
                                        accs[g][:, h * OC:(h + 1) * OC],
                                        xT[:, dc * TS + tt * P:
                                           dc * TS + (tt + 1) * P],
                                        wt[:],
                                        start=(dc == 0 and h == 0),
                                        stop=False)

                    # LoRA contribution accumulates into the same PSUM tiles
                    for g in range(4):
                        for h in range(2):
                            tt = 2 * g + h
                            nc.tensor.matmul(
                                accs[g][:, h * OC:(h + 1) * OC],
                                u_mT[:, tt * P:(tt + 1) * P],
                                wbsT[:, o0:o0 + OC],
                                start=False, stop=(h == 1))

                    # bias broadcast for this o-chunk
                    brow = bias_p.tile([1, OC], F32, tag="brow")
                    nc.sync.dma_start(
                        out=brow[:],
                        in_=b[o0:o0 + OC].rearrange("(a f) -> a f", a=1))
                    bps = b_ps.tile([P, OC], F32, tag="bps")
                    nc.tensor.matmul(bps[:], ones_row[:], brow[:],
                                     start=True, stop=True)
                    bias_sb = bias_p.tile([P, OC], F32, tag="bias")
                    nc.any.tensor_copy(bias_sb[:], bps[:])

                    for g in range(4):
                        for h in range(2):
                            tt = 2 * g + h
                            osb = out_p.tile([P, OC], F32, tag="osb",
                                             name=f"osb{oc}_{g}_{h}")
                            nc.vector.tensor_add(
                                osb[:], accs[g][:, h * OC:(h + 1) * OC],
                                bias_sb[:])
                            nc.sync.dma_start(
                                out=out[tt * P:(tt + 1) * P, o0:o0 + OC],
                                in_=osb[:])
    nc.finalize()
    return nc


_NC = None


def _get_nc():
    global _NC
    if _NC is None:
        _NC = _build()
    return _NC


class _Runner:
    """Cached PJRT executable for the SPMD bass kernel.

    Mirrors concourse.bass2jax.run_bass_via_pjrt's multi-core path but
    keeps the jitted shard_map callable alive across invocations so
    repeated kernel() calls skip retrace/recompile.
    """

    # inputs sharded over the token dim; everything else replicated
    _CORE_SHARDED = {"x", "tl"}

    def __init__(self):
        import jax
        import concourse.mybir as mybir_
        from concourse import bass2jax

        bass2jax.install_neuronx_cc_hook()
        self._bass2jax = bass2jax
        nc = _get_nc()
        self.nc = nc

        partition_name = (nc.partition_id_tensor.name
                          if nc.partition_id_tensor else None)
        in_names, out_names, out_avals, zero_outs = [], [], [], []
        for alloc in nc.m.functions[0].allocations:
            if not isinstance(alloc, mybir_.MemoryLocationSet):
                continue
            name = alloc.memorylocations[0].name
            if alloc.kind == "ExternalInput":
                if name != partition_name:
                    in_names.append(name)
            elif alloc.kind == "ExternalOutput":
                shape = tuple(alloc.tensor_shape)
                dtype = mybir_.dt.np(alloc.dtype)
                out_names.append(name)
                out_avals.append(jax.core.ShapedArray(shape, dtype))
                zero_outs.append((shape, dtype))
        self.in_names = list(in_names)
        self.out_names = out_names
        self.out_avals = out_avals
        n_params = len(in_names)
        all_in_names = in_names + out_names
        if partition_name is not None:
            all_in_names.append(partition_name)

        from jax.experimental.shard_map import shard_map
        from jax.sharding import Mesh, NamedSharding, PartitionSpec

        devices = jax.devices()[:NCORES]
        assert len(devices) == NCORES, devices
        mesh = Mesh(np.asarray(devices), ("core",))
        self.mesh = mesh

        def spec_for(name):
            return (PartitionSpec("core") if name in self._CORE_SHARDED
                    else PartitionSpec())

        in_specs = tuple(spec_for(n) for n in in_names) + \
            (PartitionSpec("core"),) * len(out_names)
        out_specs = (PartitionSpec("core"),) * len(out_names)
        self.in_shardings = [NamedSharding(mesh, spec_for(n))
                             for n in in_names]
        self.out_sharding = NamedSharding(mesh, PartitionSpec("core"))

        def _body(*args):
            operands = list(args)
            if partition_name is not None:
                operands.append(bass2jax.partition_id_tensor())
            outs = bass2jax._bass_exec_p.bind(
                *operands,
                out_avals=tuple(out_avals),
                in_names=tuple(all_in_names),
                out_names=tuple(out_names),
                lowering_input_output_aliases=(),
                sim_require_finite=True,
                sim_require_nnan=True,
                nc=nc,
            )
            return tuple(outs)

        self._fn = jax.jit(
            shard_map(_body, mesh=mesh, in_specs=in_specs,
                      out_specs=out_specs, check_rep=False),
            keep_unused=True)
        # resident zero operands for the NEFF's output-tensor inputs (the
        # kernel writes every output element, so contents don't matter and
        # the same device buffers are reused every call)
        import jax
        self._scratch_dev = [
            jax.device_put(
                np.zeros((NCORES * a.shape[0], *a.shape[1:]), a.dtype),
                self.out_sharding)
            for a in out_avals
        ]

    def put_inputs(self, by_name):
        import jax
        out = []
        for name, sharding in zip(self.in_names, self.in_shardings):
            out.append(jax.device_put(by_name[name], sharding))
        return out

    def run_device(self, dev_args):
        """dev_args: device arrays in in_names order. Returns jax arrays."""
        return self._fn(*dev_args, *self._scratch_dev)

    def run(self, by_name):
        outs = self.run_device(self.put_inputs(by_name))
        host = [np.asarray(o) for o in outs]
        return {n: h for n, h in zip(self.out_names, host)}


_RUNNER = None


def _get_runner():
    global _RUNNER
    if _RUNNER is None:
        _RUNNER = _Runner()
    return _RUNNER


def _global_inputs(x, W_base, b_base, WA, WB, scaling, token_lora):
    """Full-size (global) arrays keyed by DRAM-parameter name."""
    return {
        "x": np.ascontiguousarray(np.asarray(x, dtype=np.float32)),
        "w": np.ascontiguousarray(np.asarray(W_base, dtype=np.float32)),
        "b": np.ascontiguousarray(np.asarray(b_base, dtype=np.float32)),
        "wa": np.ascontiguousarray(
            np.asarray(WA, dtype=np.float32).reshape(LR, D)),
        "wb": np.ascontiguousarray(np.asarray(WB, dtype=np.float32)),
        "scal": np.ascontiguousarray(np.asarray(scaling, dtype=np.float32)),
        "tl": np.ascontiguousarray(np.asarray(token_lora, dtype=np.int32)),
    }


def kernel(x, W_base, b_base, WA, WB, scaling, token_lora):
    by_name = _global_inputs(x, W_base, b_base, WA, WB, scaling, token_lora)
    try:
        res = _get_runner().run(by_name)
        return res["out"]
    except Exception:
        # robust fallback through the library SPMD path
        from concourse.bass_utils import run_bass_kernel_spmd

        nc = _get_nc()
        in_maps = []
        for c in range(NCORES):
            in_maps.append({
                "x": by_name["x"][c * TS:(c + 1) * TS],
                "w": by_name["w"],
                "b": by_name["b"],
                "wa": by_name["wa"],
                "wb": by_name["wb"],
                "scal": by_name["scal"],
                "tl": by_name["tl"][c * TS:(c + 1) * TS],
            })
        res = run_bass_kernel_spmd(nc, in_maps, core_ids=list(range(NCORES)))
        return np.concatenate(
            [res.results[c]["out"] for c in range(NCORES)], axis=0)
